# revision 29
# baseline (speedup 1.0000x reference)
"""Trainium2 Bass kernel for the Capsule routing layer (nn_Capsule_49658411876931).

Math (see reference):
    u_hat[b,j,i,d] = sum_k W[j,i,d,k] * x[b,i,k]
    b0 = 0
    for r in 0..2:
        c = softmax(b, axis=j)
        s[b,j,d] = sum_i c[b,j,i] u_hat[b,j,i,d]
        v = squash(s)  (over d)
        if r < 2: b += sum_d u_hat[b,j,i,d] v[b,j,d]
    return v  [B, J, D]

Sharding: input-capsule axis I=2048 split over 8 cores (I_LOC=256). W is
I-sharded (2.1 MB/core in bf16 instead of 33 MB replicated). Softmax over J is
core-local; the only cross-core communication is an AllReduce of the partial
s [B, J*D] = 32 KB (bf16) per routing iteration (plus a tiny warm-up
AllReduce issued during phase 1 to absorb the collective's cold-start cost).

Per-core layouts (P = SBUF partition index):
  i_local = g*16 + r*4 + c   (g in 0..15, r,c in 0..3)
  W / x operands     : P = 32*r + k
  u_hat "C" tensor   : [P = 32*c + b, free = (g, r, d, j)]  bf16
  b-logits / c       : [P = 32*c + b, free = (g, r, j)]
u_hat is computed with 16-way tile_position-packed PE matmuls (stationary
x_i [k=8, b=32] at array tile (r,c), moving W_i [k=8, jd=512]), all in
bf16 (fp32 matmuls lower to two array passes - 4x slower). W streams in
per-(g,r) chunks, alternating the sync/gpsimd DMA queues, with the
matmuls chasing the chunks (no global barrier on W).

The i-contraction s = sum_i c*u_hat runs on the PE as 64 accumulating
matmuls (one per (g,r,q) slice of C) against a 0/1 "collapse the 4
c-strips" selector stationary; for iteration 0 the softmax coefficients
are the constant 1/J, so the matmuls read C directly through a
(1/J)-scaled selector and no elementwise pass is needed at all. The
agreement d-contraction runs as a log2 tree of bf16 2x-mode
tensor_tensor adds (tensor_reduce only has a 1x micro-op), and the
agreement -> logit update -> softmax -> next-iteration product -> s
matmuls are interleaved per 16-slice block so PE/ACT work hides under
the DVE stream.
"""

import numpy as np
import ml_dtypes

import concourse.bass as bass
import concourse.tile as tile
from concourse import bacc, mybir
from concourse.bass_utils import run_bass_kernel_spmd

F32 = mybir.dt.float32
BF16 = mybir.dt.bfloat16
U32 = mybir.dt.uint32
Alu = mybir.AluOpType
Act = mybir.ActivationFunctionType

B, I, K = 32, 2048, 8
J, D = 32, 16
JD = J * D                     # 512
NCORES = 8
I_LOC = I // NCORES            # 256
NG = I_LOC // 16               # 16 groups of 16 input capsules per core
GB = 4                         # groups per routing block
NBLK = NG // GB                # 4 routing blocks (16 i-slices each)
ROUTINGS = 3
EPS = 1e-7

_CACHE = {}


def _build():
    nc = bacc.Bacc("TRN2", target_bir_lowering=False, debug=False, num_devices=NCORES)

    wt_in = nc.dram_tensor("wt", [NG, 4, 8, 4, JD], BF16, kind="ExternalInput")
    xs_in = nc.dram_tensor("xs", [4, 8, NG, 4, B], BF16, kind="ExternalInput")
    v_out = nc.dram_tensor("v", [B, J, D], F32, kind="ExternalOutput")

    # Selector constants for cross-partition PE ops:
    #   sel[p, b'] = 1 iff p % 32 == b'    (collapse the 4 c-strips)
    #   sel32 = sel / 32                   (fold in the uniform iter-0 softmax)
    #   selT[b, p] = sel^T                 (replicate v over the 4 c-strips)
    sel_np = np.zeros((128, B), np.float32)
    sel_np[np.arange(128), np.arange(128) % B] = 1.0
    selpack = np.zeros((128, 2 * B + 128), np.float32)
    selpack[:, 0:B] = sel_np
    selpack[:, B : 2 * B] = sel_np / 32.0
    selpack[0:B, 2 * B :] = sel_np.T
    sel_dram = nc.inline_tensor(selpack.astype(ml_dtypes.bfloat16), "selpack")

    with tile.TileContext(nc) as tc:
        with (
            tc.tile_pool(name="persist", bufs=1) as pp,
            tc.tile_pool(name="small", bufs=1) as sp,
            tc.tile_pool(name="dram", bufs=1, space="DRAM") as dp,
            tc.tile_pool(name="spsum", bufs=1, space="PSUM") as ssp,
        ):
            # ---- persistent SBUF tensors ----
            xs = pp.tile([128, NG, 4, B], BF16)         # x stationary, rows 32r+k
            C = pp.tile([128, NG, 4, D, J], BF16)       # u_hat [(g r) slices]
            bl = pp.tile([128, NG, 4, J], F32)          # routing logits
            c_sb = pp.tile([128, NG, 4, J], BF16)       # softmax coefficients
            p_t = pp.tile([128, NG, 4, J], F32)         # exp(b)
            selc = pp.tile([128, 2 * B + 128], BF16)
            v_rep = pp.tile([128, D, J], BF16)          # v replicated over c-strips
            v_fb = pp.tile([B, D, J], BF16)             # v as bf16 (vr matmul moving)
            warm = sp.tile([B, D * J], BF16, tag="warm")

            sel = selc[:, 0:B]
            sel32 = selc[:, B : 2 * B]
            selT = selc[0:B, 2 * B :]

            # s accumulator / v-replicate PSUM banks, persist across the kernel
            s_ps = ssp.tile([B, D * J], F32)
            vr_ps = ssp.tile([128, D * J], F32)

            nc.sync.dma_start(selc[:], sel_dram[:])
            for r in range(4):
                nc.sync.dma_start(xs[32 * r : 32 * r + 8], xs_in[r])
            nc.vector.memset(bl[:], 0.0)
            nc.vector.memset(warm[:], 0.0)
            # Funnel the initial-load waits through one barrier so the first
            # matmuls don't exceed the per-instruction sync-wait budget.
            # W is NOT behind the barrier: its chunks stream in during
            # phase 1 and the matmuls chase them.
            tc.strict_bb_all_engine_barrier()
            # Warm-up AllReduce issued right after the barrier: the
            # collective path's ~65us one-time init runs on the CC cores
            # concurrently with phase 1 instead of gating iteration 0's real
            # AllReduce. (Issued after the barrier because collective_compute
            # holds the gpsimd queue, and the barrier waits on it.) The
            # payload is junk and never read back.
            w_in = dp.tile([B, D * J], BF16, tag="w_in")
            w_out = dp.tile([B, D * J], BF16, tag="w_out", addr_space="Shared")
            nc.sync.dma_start(w_in[:], warm[:])
            nc.gpsimd.collective_compute(
                "AllReduce",
                Alu.add,
                replica_groups=[list(range(NCORES))],
                ins=[w_in.opt()],
                outs=[w_out.opt()],
            )

            # ---- phase 1: u_hat + iteration-0 s accumulation ----
            def s0_half(h):
                # two accumulating s0 matmuls for r-slice half h (0..2*NG-1)
                g, rh = divmod(h, 2)
                for r2 in range(2):
                    r = 2 * rh + r2
                    kk = 2 * h + r2
                    nc.tensor.matmul(
                        s_ps[:],
                        sel32,
                        C[:, g, r].rearrange("p d j -> p (d j)"),
                        start=(kk == 0),
                        stop=(kk == 4 * NG - 1),
                    )

            with (
                tc.tile_pool(name="wpool", bufs=1) as wp,
                tc.tile_pool(name="psum1", bufs=2, space="PSUM") as ps1,
            ):
                wt = wp.tile([128, NG, 4, JD], BF16)    # W moving, rows 32r+k

                # W chunk DMAs, alternating queues; issue a few groups ahead
                def w_dma(g):
                    # sync/scalar only: gpsimd's queue is blocked behind the
                    # warm-up collective until it completes
                    for r in range(4):
                        eng = nc.sync if (g * 4 + r) % 2 == 0 else nc.scalar
                        eng.dma_start(wt[32 * r : 32 * r + 8, g], wt_in[g, r])

                for g in range(2):
                    w_dma(g)
                nsub = 0
                for g in range(NG):
                    if g + 2 < NG:
                        w_dma(g + 2)
                    for rh in range(2):
                        ps = ps1.tile([128, 2, JD], F32, tag="ps")
                        for r2 in range(2):
                            r = 2 * rh + r2
                            pb = 32 * r
                            for c in range(4):
                                nc.tensor.matmul(
                                    ps[32 * c : 32 * c + 32, r2, :],
                                    xs[pb : pb + 8, g, c, :],
                                    wt[pb : pb + 8, g, c, :],
                                    tile_position=(32 * r, 32 * c),
                                )
                        src = ps.rearrange("p r (j d) -> p r d j", j=J, d=D)
                        dst = C[:, g, 2 * rh : 2 * rh + 2]
                        if nsub % 2 == 0:
                            nc.scalar.copy(dst, src)
                        else:
                            nc.vector.tensor_copy(dst, src)
                        nsub += 1
                        if nsub >= 4:
                            s0_half(nsub - 4)
                for h in range(2 * NG - 3, 2 * NG):
                    s0_half(h)

            # ---- routing ----
            with tc.tile_pool(name="blk", bufs=2) as bp:
                for it in range(ROUTINGS):
                    # s_ps holds this iteration's partial s; AllReduce in bf16
                    s_loc = sp.tile([B, D * J], BF16, tag="s_loc")
                    nc.scalar.copy(s_loc[:], s_ps[:])
                    cc_in = dp.tile([B, D * J], BF16, tag="cc_in")
                    cc_out = dp.tile(
                        [B, D * J], BF16, tag="cc_out", addr_space="Shared"
                    )
                    s_gb = sp.tile([B, D, J], BF16, tag="s_gb")
                    nc.gpsimd.dma_start(cc_in[:], s_loc[:])
                    nc.gpsimd.collective_compute(
                        "AllReduce",
                        Alu.add,
                        replica_groups=[list(range(NCORES))],
                        ins=[cc_in.opt()],
                        outs=[cc_out.opt()],
                    )
                    nc.gpsimd.dma_start(
                        s_gb.rearrange("b d j -> b (d j)"), cc_out[:]
                    )

                    # ---- squash on [B, D, J] (all cores redundantly) ----
                    s_glob = sp.tile([B, D, J], F32, tag="s_glob")
                    nc.scalar.copy(s_glob[:], s_gb[:])
                    sq = sp.tile([B, D, J], F32, tag="sq")
                    nc.scalar.activation(sq[:], s_glob[:], Act.Square)
                    n2 = sp.tile([B, J], F32, tag="n2")
                    nc.vector.tensor_reduce(
                        n2[:],
                        sq.rearrange("b d j -> b j d"),
                        axis=mybir.AxisListType.X,
                        op=Alu.add,
                    )
                    # factor = n2 / (1 + n2) / sqrt(n2 + eps)
                    n2e = sp.tile([B, J], F32, tag="n2e")
                    nc.vector.tensor_scalar_add(n2e[:], n2[:], EPS)
                    sd = sp.tile([B, J], F32, tag="sd")
                    nc.scalar.activation(sd[:], n2e[:], Act.Sqrt)
                    tmp = sp.tile([B, J], F32, tag="tmp")
                    nc.vector.tensor_scalar_add(tmp[:], n2[:], 1.0)
                    nc.vector.tensor_tensor(tmp[:], tmp[:], sd[:], op=Alu.mult)
                    fac = sp.tile([B, J], F32, tag="fac")
                    nc.vector.reciprocal(fac[:], tmp[:])
                    nc.vector.tensor_tensor(fac[:], fac[:], n2[:], op=Alu.mult)
                    v_f = sp.tile([B, D, J], F32, tag="v_f")
                    nc.vector.tensor_tensor(
                        v_f[:],
                        s_glob[:],
                        fac[:, None, :].broadcast_to([B, D, J]),
                        op=Alu.mult,
                    )

                    if it < ROUTINGS - 1:
                        # replicate v over the 4 c-strips via PE
                        nc.scalar.copy(v_fb[:], v_f[:])
                        nc.tensor.matmul(
                            vr_ps[:], selT, v_fb.rearrange("b d j -> b (d j)")
                        )
                        nc.scalar.copy(v_rep.rearrange("p d j -> p (d j)"), vr_ps[:])
                        # pass A per block: agreement (log2 tree over d) and
                        # logit update. DVE handles blocks 0-2; the otherwise
                        # idle GpSimd engine runs block 3's whole chain in
                        # parallel. ACT exp issues per block as bl lands.
                        def chain_A(blk, eng):
                            g0 = blk * GB
                            gs = slice(g0, g0 + GB)
                            Cb = C[:, gs]
                            pi2 = bp.tile([128, GB, 4, D, J], BF16, tag="pi")
                            eng.tensor_tensor(
                                pi2[:],
                                Cb,
                                v_rep[:, None, None, :, :].broadcast_to(
                                    [128, GB, 4, D, J]
                                ),
                                op=Alu.mult,
                            )
                            t8 = bp.tile([128, GB, 4, 8, J], BF16, tag="t8")
                            eng.tensor_tensor(
                                t8[:], pi2[:, :, :, 0:8, :], pi2[:, :, :, 8:16, :],
                                op=Alu.add,
                            )
                            t4 = bp.tile([128, GB, 4, 4, J], BF16, tag="t4")
                            eng.tensor_tensor(
                                t4[:], t8[:, :, :, 0:4, :], t8[:, :, :, 4:8, :],
                                op=Alu.add,
                            )
                            t2 = bp.tile([128, GB, 4, 2, J], BF16, tag="t2")
                            eng.tensor_tensor(
                                t2[:], t4[:, :, :, 0:2, :], t4[:, :, :, 2:4, :],
                                op=Alu.add,
                            )
                            t1 = bp.tile([128, GB, 4, J], F32, tag="t1")
                            eng.tensor_tensor(
                                t1[:], t2[:, :, :, 0, :], t2[:, :, :, 1, :],
                                op=Alu.add,
                            )
                            eng.tensor_add(bl[:, gs], bl[:, gs], t1[:])
                            nc.scalar.activation(p_t[:, gs], bl[:, gs], Act.Exp)

                        for blk in range(NBLK):
                            chain_A(blk, nc.vector)
                        # pass B: softmax finish (merged over all blocks),
                        # then per-block next-iteration product + s matmuls
                        S = sp.tile([128, NG, 4], F32, tag="S")
                        nc.vector.tensor_reduce(
                            S[:], p_t[:], axis=mybir.AxisListType.X, op=Alu.add
                        )
                        Sr = sp.tile([128, NG, 4], F32, tag="Sr")
                        nc.vector.reciprocal(Sr[:], S[:])
                        nc.vector.tensor_tensor(
                            c_sb[:],
                            p_t[:],
                            Sr[:, :, :, None].broadcast_to([128, NG, 4, J]),
                            op=Alu.mult,
                        )
                        for blk in range(NBLK):
                            g0 = blk * GB
                            gs = slice(g0, g0 + GB)
                            Cb = C[:, gs]
                            pi = bp.tile([128, GB, 4, D, J], BF16, tag="pi")
                            nc.vector.tensor_tensor(
                                pi[:],
                                Cb,
                                c_sb[:, gs, :, None, :].broadcast_to(
                                    [128, GB, 4, D, J]
                                ),
                                op=Alu.mult,
                            )
                            for gr in range(GB * 4):
                                g2, r = divmod(gr, 4)
                                kk = blk * GB * 4 + gr
                                nc.tensor.matmul(
                                    s_ps[:],
                                    sel,
                                    pi[:, g2, r].rearrange("p d j -> p (d j)"),
                                    start=(kk == 0),
                                    stop=(kk == 4 * NG - 1),
                                )
                    else:
                        # final output: reorder (d, j) -> (j, d) and store
                        v_jd = sp.tile([B, J, D], F32, tag="v_jd")
                        nc.vector.tensor_copy(
                            v_jd[:], v_f.rearrange("b d j -> b j d")
                        )
                        nc.sync.dma_start(v_out[:], v_jd[:])

    nc.compile()
    return nc


def _prep_inputs(x, W):
    """Per-core host-side sharding + layout prep (bf16)."""
    in_maps = []
    for m in range(NCORES):
        lo, hi = m * I_LOC, (m + 1) * I_LOC
        Wc = W[:, lo:hi]                       # [J, 256, D, K]
        Wc = Wc.reshape(J, NG, 4, 4, D, K)     # i = g*16 + r*4 + c
        # -> [g, r, k, c, j, d]
        wt = np.ascontiguousarray(Wc.transpose(1, 2, 5, 3, 0, 4)).reshape(
            NG, 4, 8, 4, JD
        )
        xc = x[:, lo:hi, :].reshape(B, NG, 4, 4, K)
        xs = np.ascontiguousarray(xc.transpose(2, 4, 1, 3, 0))  # [r, k, g, c, b]
        in_maps.append(
            {
                "wt": wt.astype(ml_dtypes.bfloat16),
                "xs": xs.astype(ml_dtypes.bfloat16),
            }
        )
    return in_maps


def run(inputs, trace=False):
    if "nc" not in _CACHE:
        _CACHE["nc"] = _build()
    nc = _CACHE["nc"]
    in_maps = _prep_inputs(inputs["x"], inputs["W"])
    bkr = run_bass_kernel_spmd(
        nc, in_maps, core_ids=list(range(NCORES)), trace=trace
    )
    out = bkr.results[0]["v"].astype(np.float32)
    return out, bkr


def kernel(x, W):
    out, _ = run({"x": np.asarray(x), "W": np.asarray(W)})
    return out


# revision 30
# speedup vs baseline: 1.2449x; 1.2449x over previous
"""Trainium2 Bass kernel for the Capsule routing layer (nn_Capsule_49658411876931).

Math (see reference):
    u_hat[b,j,i,d] = sum_k W[j,i,d,k] * x[b,i,k]
    b0 = 0
    for r in 0..2:
        c = softmax(b, axis=j)
        s[b,j,d] = sum_i c[b,j,i] u_hat[b,j,i,d]
        v = squash(s)  (over d)
        if r < 2: b += sum_d u_hat[b,j,i,d] v[b,j,d]
    return v  [B, J, D]

Sharding: input-capsule axis I=2048 split over 8 cores (I_LOC=256). W is
I-sharded (2.1 MB/core in bf16 instead of 33 MB replicated). Softmax over J is
core-local; the only cross-core communication is an AllReduce of the partial
s [B, J*D] = 32 KB (bf16) per routing iteration (plus a tiny warm-up
AllReduce issued during phase 1 to absorb the collective's cold-start cost).

Per-core layouts (P = SBUF partition index):
  i_local = g*16 + r*4 + c   (g in 0..15, r,c in 0..3)
  W / x operands     : P = 32*r + k
  u_hat "C" tensor   : [P = 32*c + b, free = (g, r, d, j)]  bf16
  b-logits / c       : [P = 32*c + b, free = (g, r, j)]
u_hat is computed with 16-way tile_position-packed PE matmuls (stationary
x_i [k=8, b=32] at array tile (r,c), moving W_i [k=8, jd=512]), all in
bf16 (fp32 matmuls lower to two array passes - 4x slower). W streams in
per-(g,r) chunks, alternating the sync/gpsimd DMA queues, with the
matmuls chasing the chunks (no global barrier on W).

The i-contraction s = sum_i c*u_hat runs on the PE as 64 accumulating
matmuls (one per (g,r,q) slice of C) against a 0/1 "collapse the 4
c-strips" selector stationary; for iteration 0 the softmax coefficients
are the constant 1/J, so the matmuls read C directly through a
(1/J)-scaled selector and no elementwise pass is needed at all. The
agreement d-contraction runs as a log2 tree of bf16 2x-mode
tensor_tensor adds (tensor_reduce only has a 1x micro-op), and the
agreement -> logit update -> softmax -> next-iteration product -> s
matmuls are interleaved per 16-slice block so PE/ACT work hides under
the DVE stream.
"""

import numpy as np
import ml_dtypes

import concourse.bass as bass
import concourse.tile as tile
from concourse import bacc, mybir
from concourse.bass_utils import run_bass_kernel_spmd

F32 = mybir.dt.float32
BF16 = mybir.dt.bfloat16
U32 = mybir.dt.uint32
Alu = mybir.AluOpType
Act = mybir.ActivationFunctionType

B, I, K = 32, 2048, 8
J, D = 32, 16
JD = J * D                     # 512
NCORES = 8
I_LOC = I // NCORES            # 256
NG = I_LOC // 16               # 16 groups of 16 input capsules per core
GB = 4                         # groups per routing block
NBLK = NG // GB                # 4 routing blocks (16 i-slices each)
ROUTINGS = 3
EPS = 1e-7

_CACHE = {}


def _build():
    nc = bacc.Bacc("TRN2", target_bir_lowering=False, debug=False, num_devices=NCORES)

    wt_in = nc.dram_tensor("wt", [NG, 4, 8, 4, JD], BF16, kind="ExternalInput")
    xs_in = nc.dram_tensor("xs", [4, 8, NG, 4, B], BF16, kind="ExternalInput")
    v_out = nc.dram_tensor("v", [B, J, D], F32, kind="ExternalOutput")

    # Selector constants for cross-partition PE ops:
    #   sel[p, b'] = 1 iff p % 32 == b'    (collapse the 4 c-strips)
    #   sel32 = sel / 32                   (fold in the uniform iter-0 softmax)
    #   selT[b, p] = sel^T                 (replicate v over the 4 c-strips)
    sel_np = np.zeros((128, B), np.float32)
    sel_np[np.arange(128), np.arange(128) % B] = 1.0
    selpack = np.zeros((128, 2 * B + 128), np.float32)
    selpack[:, 0:B] = sel_np
    selpack[:, B : 2 * B] = sel_np / 32.0
    selpack[0:B, 2 * B :] = sel_np.T
    sel_dram = nc.inline_tensor(selpack.astype(ml_dtypes.bfloat16), "selpack")

    with tile.TileContext(nc) as tc:
        with (
            tc.tile_pool(name="persist", bufs=1) as pp,
            tc.tile_pool(name="small", bufs=1) as sp,
            tc.tile_pool(name="dram", bufs=1, space="DRAM") as dp,
            tc.tile_pool(name="spsum", bufs=1, space="PSUM") as ssp,
        ):
            # ---- persistent SBUF tensors ----
            xs = pp.tile([128, NG, 4, B], BF16)         # x stationary, rows 32r+k
            C = pp.tile([128, NG, 4, D, J], BF16)       # u_hat [(g r) slices]
            bl = pp.tile([128, NG, 4, J], F32)          # routing logits
            c_sb = pp.tile([128, NG, 4, J], BF16)       # softmax coefficients
            p_t = pp.tile([128, NG, 4, J], F32)         # exp(b)
            selc = pp.tile([128, 2 * B + 128], BF16)
            v_rep = pp.tile([128, D, J], BF16)          # v replicated over c-strips
            v_fb = pp.tile([B, D, J], BF16)             # v as bf16 (vr matmul moving)
            warm = sp.tile([B, D * J], BF16, tag="warm")

            sel = selc[:, 0:B]
            sel32 = selc[:, B : 2 * B]
            selT = selc[0:B, 2 * B :]

            # s accumulator / v-replicate PSUM banks, persist across the kernel
            s_ps = ssp.tile([B, D * J], F32)
            vr_ps = ssp.tile([128, D * J], F32)

            nc.sync.dma_start(selc[:], sel_dram[:])
            for r in range(4):
                nc.sync.dma_start(xs[32 * r : 32 * r + 8], xs_in[r])
            nc.vector.memset(bl[:], 0.0)
            nc.vector.memset(warm[:], 0.0)
            # Funnel the initial-load waits through one barrier so the first
            # matmuls don't exceed the per-instruction sync-wait budget.
            # W is NOT behind the barrier: its chunks stream in during
            # phase 1 and the matmuls chase them.
            tc.strict_bb_all_engine_barrier()
            # Warm-up AllReduce issued right after the barrier: the
            # collective path's ~65us one-time init runs on the CC cores
            # concurrently with phase 1 instead of gating iteration 0's real
            # AllReduce. (Issued after the barrier because collective_compute
            # holds the gpsimd queue, and the barrier waits on it.) The
            # payload is junk and never read back.
            w_in = dp.tile([B, D * J], BF16, tag="w_in")
            w_out = dp.tile([B, D * J], BF16, tag="w_out", addr_space="Shared")
            nc.gpsimd.dma_start(w_in[:], warm[:])
            nc.gpsimd.collective_compute(
                "AllReduce",
                Alu.add,
                replica_groups=[list(range(NCORES))],
                ins=[w_in.opt()],
                outs=[w_out.opt()],
            )
            nc.gpsimd.dma_start(warm[:], w_out[:])

            # ---- phase 1: u_hat + iteration-0 s accumulation ----
            def s0_half(h):
                # two accumulating s0 matmuls for r-slice half h (0..2*NG-1)
                g, rh = divmod(h, 2)
                for r2 in range(2):
                    r = 2 * rh + r2
                    kk = 2 * h + r2
                    nc.tensor.matmul(
                        s_ps[:],
                        sel32,
                        C[:, g, r].rearrange("p d j -> p (d j)"),
                        start=(kk == 0),
                        stop=(kk == 4 * NG - 1),
                    )

            with (
                tc.tile_pool(name="wpool", bufs=1) as wp,
                tc.tile_pool(name="psum1", bufs=2, space="PSUM") as ps1,
            ):
                wt = wp.tile([128, NG, 4, JD], BF16)    # W moving, rows 32r+k

                # W chunk DMAs, alternating queues; issue a few groups ahead
                def w_dma(g):
                    # sync/scalar only: gpsimd's queue is blocked behind the
                    # warm-up collective until it completes
                    for r in range(4):
                        eng = nc.sync if (g * 4 + r) % 2 == 0 else nc.scalar
                        eng.dma_start(wt[32 * r : 32 * r + 8, g], wt_in[g, r])

                for g in range(2):
                    w_dma(g)
                nsub = 0
                for g in range(NG):
                    if g + 2 < NG:
                        w_dma(g + 2)
                    for rh in range(2):
                        ps = ps1.tile([128, 2, JD], F32, tag="ps")
                        for r2 in range(2):
                            r = 2 * rh + r2
                            pb = 32 * r
                            for c in range(4):
                                nc.tensor.matmul(
                                    ps[32 * c : 32 * c + 32, r2, :],
                                    xs[pb : pb + 8, g, c, :],
                                    wt[pb : pb + 8, g, c, :],
                                    tile_position=(32 * r, 32 * c),
                                )
                        src = ps.rearrange("p r (j d) -> p r d j", j=J, d=D)
                        dst = C[:, g, 2 * rh : 2 * rh + 2]
                        if nsub % 2 == 0:
                            nc.scalar.copy(dst, src)
                        else:
                            nc.vector.tensor_copy(dst, src)
                        nsub += 1
                        if nsub >= 4:
                            s0_half(nsub - 4)
                for h in range(2 * NG - 3, 2 * NG):
                    s0_half(h)

            # ---- routing ----
            with tc.tile_pool(name="blk", bufs=2) as bp:
                for it in range(ROUTINGS):
                    # s_ps holds this iteration's partial s; AllReduce in bf16
                    s_loc = sp.tile([B, D * J], BF16, tag="s_loc")
                    nc.scalar.copy(s_loc[:], s_ps[:])
                    cc_in = dp.tile([B, D * J], BF16, tag="cc_in")
                    cc_out = dp.tile(
                        [B, D * J], BF16, tag="cc_out", addr_space="Shared"
                    )
                    s_gb = sp.tile([B, D, J], BF16, tag="s_gb")
                    nc.gpsimd.dma_start(cc_in[:], s_loc[:])
                    nc.gpsimd.collective_compute(
                        "AllReduce",
                        Alu.add,
                        replica_groups=[list(range(NCORES))],
                        ins=[cc_in.opt()],
                        outs=[cc_out.opt()],
                    )
                    nc.gpsimd.dma_start(
                        s_gb.rearrange("b d j -> b (d j)"), cc_out[:]
                    )

                    # ---- squash on [B, D, J] (all cores redundantly) ----
                    s_glob = sp.tile([B, D, J], F32, tag="s_glob")
                    nc.scalar.copy(s_glob[:], s_gb[:])
                    sq = sp.tile([B, D, J], F32, tag="sq")
                    nc.scalar.activation(sq[:], s_glob[:], Act.Square)
                    n2 = sp.tile([B, J], F32, tag="n2")
                    nc.vector.tensor_reduce(
                        n2[:],
                        sq.rearrange("b d j -> b j d"),
                        axis=mybir.AxisListType.X,
                        op=Alu.add,
                    )
                    # factor = n2 / (1 + n2) / sqrt(n2 + eps)
                    n2e = sp.tile([B, J], F32, tag="n2e")
                    nc.vector.tensor_scalar_add(n2e[:], n2[:], EPS)
                    sd = sp.tile([B, J], F32, tag="sd")
                    nc.scalar.activation(sd[:], n2e[:], Act.Sqrt)
                    tmp = sp.tile([B, J], F32, tag="tmp")
                    nc.vector.tensor_scalar_add(tmp[:], n2[:], 1.0)
                    nc.vector.tensor_tensor(tmp[:], tmp[:], sd[:], op=Alu.mult)
                    fac = sp.tile([B, J], F32, tag="fac")
                    nc.vector.reciprocal(fac[:], tmp[:])
                    nc.vector.tensor_tensor(fac[:], fac[:], n2[:], op=Alu.mult)
                    v_f = sp.tile([B, D, J], F32, tag="v_f")
                    nc.vector.tensor_tensor(
                        v_f[:],
                        s_glob[:],
                        fac[:, None, :].broadcast_to([B, D, J]),
                        op=Alu.mult,
                    )

                    if it < ROUTINGS - 1:
                        # replicate v over the 4 c-strips via PE
                        nc.scalar.copy(v_fb[:], v_f[:])
                        nc.tensor.matmul(
                            vr_ps[:], selT, v_fb.rearrange("b d j -> b (d j)")
                        )
                        nc.scalar.copy(v_rep.rearrange("p d j -> p (d j)"), vr_ps[:])
                        # pass A per block: agreement (log2 tree over d) and
                        # logit update. DVE handles blocks 0-2; the otherwise
                        # idle GpSimd engine runs block 3's whole chain in
                        # parallel. ACT exp issues per block as bl lands.
                        def chain_A(blk, eng):
                            g0 = blk * GB
                            gs = slice(g0, g0 + GB)
                            Cb = C[:, gs]
                            pi2 = bp.tile([128, GB, 4, D, J], BF16, tag="pi")
                            eng.tensor_tensor(
                                pi2[:],
                                Cb,
                                v_rep[:, None, None, :, :].broadcast_to(
                                    [128, GB, 4, D, J]
                                ),
                                op=Alu.mult,
                            )
                            t8 = bp.tile([128, GB, 4, 8, J], BF16, tag="t8")
                            eng.tensor_tensor(
                                t8[:], pi2[:, :, :, 0:8, :], pi2[:, :, :, 8:16, :],
                                op=Alu.add,
                            )
                            t4 = bp.tile([128, GB, 4, 4, J], BF16, tag="t4")
                            eng.tensor_tensor(
                                t4[:], t8[:, :, :, 0:4, :], t8[:, :, :, 4:8, :],
                                op=Alu.add,
                            )
                            t2 = bp.tile([128, GB, 4, 2, J], BF16, tag="t2")
                            eng.tensor_tensor(
                                t2[:], t4[:, :, :, 0:2, :], t4[:, :, :, 2:4, :],
                                op=Alu.add,
                            )
                            t1 = bp.tile([128, GB, 4, J], F32, tag="t1")
                            eng.tensor_tensor(
                                t1[:], t2[:, :, :, 0, :], t2[:, :, :, 1, :],
                                op=Alu.add,
                            )
                            eng.tensor_add(bl[:, gs], bl[:, gs], t1[:])
                            nc.scalar.activation(p_t[:, gs], bl[:, gs], Act.Exp)

                        for blk in range(NBLK):
                            chain_A(blk, nc.vector)
                        # pass B: softmax finish (merged over all blocks),
                        # then per-block next-iteration product + s matmuls
                        S = sp.tile([128, NG, 4], F32, tag="S")
                        nc.vector.tensor_reduce(
                            S[:], p_t[:], axis=mybir.AxisListType.X, op=Alu.add
                        )
                        Sr = sp.tile([128, NG, 4], F32, tag="Sr")
                        nc.vector.reciprocal(Sr[:], S[:])
                        nc.vector.tensor_tensor(
                            c_sb[:],
                            p_t[:],
                            Sr[:, :, :, None].broadcast_to([128, NG, 4, J]),
                            op=Alu.mult,
                        )
                        for blk in range(NBLK):
                            g0 = blk * GB
                            gs = slice(g0, g0 + GB)
                            Cb = C[:, gs]
                            pi = bp.tile([128, GB, 4, D, J], BF16, tag="pi")
                            nc.vector.tensor_tensor(
                                pi[:],
                                Cb,
                                c_sb[:, gs, :, None, :].broadcast_to(
                                    [128, GB, 4, D, J]
                                ),
                                op=Alu.mult,
                            )
                            for gr in range(GB * 4):
                                g2, r = divmod(gr, 4)
                                kk = blk * GB * 4 + gr
                                nc.tensor.matmul(
                                    s_ps[:],
                                    sel,
                                    pi[:, g2, r].rearrange("p d j -> p (d j)"),
                                    start=(kk == 0),
                                    stop=(kk == 4 * NG - 1),
                                )
                    else:
                        # final output: reorder (d, j) -> (j, d) and store
                        v_jd = sp.tile([B, J, D], F32, tag="v_jd")
                        nc.vector.tensor_copy(
                            v_jd[:], v_f.rearrange("b d j -> b j d")
                        )
                        nc.sync.dma_start(v_out[:], v_jd[:])

    nc.compile()
    return nc


def _prep_inputs(x, W):
    """Per-core host-side sharding + layout prep (bf16)."""
    in_maps = []
    for m in range(NCORES):
        lo, hi = m * I_LOC, (m + 1) * I_LOC
        Wc = W[:, lo:hi]                       # [J, 256, D, K]
        Wc = Wc.reshape(J, NG, 4, 4, D, K)     # i = g*16 + r*4 + c
        # -> [g, r, k, c, j, d]
        wt = np.ascontiguousarray(Wc.transpose(1, 2, 5, 3, 0, 4)).reshape(
            NG, 4, 8, 4, JD
        )
        xc = x[:, lo:hi, :].reshape(B, NG, 4, 4, K)
        xs = np.ascontiguousarray(xc.transpose(2, 4, 1, 3, 0))  # [r, k, g, c, b]
        in_maps.append(
            {
                "wt": wt.astype(ml_dtypes.bfloat16),
                "xs": xs.astype(ml_dtypes.bfloat16),
            }
        )
    return in_maps


def run(inputs, trace=False):
    if "nc" not in _CACHE:
        _CACHE["nc"] = _build()
    nc = _CACHE["nc"]
    in_maps = _prep_inputs(inputs["x"], inputs["W"])
    bkr = run_bass_kernel_spmd(
        nc, in_maps, core_ids=list(range(NCORES)), trace=trace
    )
    out = bkr.results[0]["v"].astype(np.float32)
    return out, bkr


def kernel(x, W):
    out, _ = run({"x": np.asarray(x), "W": np.asarray(W)})
    return out
